# revision 6
# baseline (speedup 1.0000x reference)
"""External Attention (nn_External_Attention) on 8 TRN2 NeuronCores.

kernel(x, Wk, Wv) -> x + Wv @ l1norm_M(softmax_N(Wk @ x))
  x  [16, 512, 4096] f32,  Wk [256, 512] f32,  Wv [512, 256] f32

Sharding: data-parallel over batch B=16 -> 2 batches per core across 8 cores.
Each core runs an identical Bass/Tile program on its batch shard; results are
concatenated on host.

Per-core pipeline (C=512, M=256, N=4096), all-bf16 on the PE:
  x is loaded HBM->SBUF as bf16 via casting SWDGE (gpsimd) DMAs -- no engine
  pass for the conversion, half the SBUF residency of f32.
  phase A (per 512-col tile j): pl = Wk^T x (PE, bf16), E = exp(pl) (ACT,
      with per-tile row-sum accumulators)
  stats: r = sum_N E, rr = 1/r (DVE), Wv' = Wv^T * rr (bf16)
  chains (all j of a batch back-to-back, so ACT Exp<->Recip table swaps are
      batched: 3 total instead of one per j):
      cs = rr^T E (PE), rcs = 1/cs (ACT raw table recip, bf16 out),
      bc = partition_broadcast(rcs) (GPSIMD), E' = E*bc (DVE, 2x bf16 mode)
  phase B (per j-pair, interleaved with phase A of the next batch to keep
      the PE and DMA continuously busy):
      po[128,1024] = Wv'^T E' (PE, bf16, two PSUM banks),
      y = po + x_bf16 (DVE), y -> HBM (HWDGE on SP)

End-to-end relative L2 error vs the fp32 reference: ~1e-3 (bf16 x add).
"""
from contextlib import ExitStack

import numpy as np

import concourse.bacc as bacc
import concourse.mybir as mybir
import concourse.tile as tile
from concourse.bass_utils import run_bass_kernel_spmd

F32 = mybir.dt.float32
BF16 = mybir.dt.bfloat16
AF = mybir.ActivationFunctionType
ALU = mybir.AluOpType
AX = mybir.AxisListType

B, C, M, N = 16, 512, 256, 4096
NCORES = 8
BPC = B // NCORES
NT = 512
NJ = N // NT          # 8 column tiles
KC = C // 128         # 4
KM = M // 128         # 2
XH = 1024             # x load chunk width (one chunk covers 2 j tiles)
NH = N // XH          # 4


def _build(nc):
    x_d = nc.dram_tensor("x", [BPC, KC, 128, N], F32, kind="ExternalInput").ap()
    wkT_d = nc.dram_tensor("wkT", [C, M], F32, kind="ExternalInput").ap()
    wvT_d = nc.dram_tensor("wvT", [M, C], F32, kind="ExternalInput").ap()
    y_d = nc.dram_tensor("y", [BPC, KC, 128, N], F32, kind="ExternalOutput").ap()

    with tile.TileContext(nc) as tc, ExitStack() as ctx:
        wpool = ctx.enter_context(tc.tile_pool(name="w", bufs=1))
        xpool = ctx.enter_context(tc.tile_pool(name="xp", bufs=2 * NH))
        epool = ctx.enter_context(tc.tile_pool(name="ep", bufs=2 * KM))
        eppool = ctx.enter_context(tc.tile_pool(name="epp", bufs=18))
        spool = ctx.enter_context(tc.tile_pool(name="sp", bufs=4))
        wvp_pool = ctx.enter_context(tc.tile_pool(name="wvp", bufs=2 * KM))
        ypool = ctx.enter_context(tc.tile_pool(name="yp", bufs=4))
        bcpool = ctx.enter_context(tc.tile_pool(name="bcp", bufs=10))
        ps_l = ctx.enter_context(tc.tile_pool(name="ps_l", bufs=2, space="PSUM"))
        ps_cs = ctx.enter_context(tc.tile_pool(name="ps_cs", bufs=2, space="PSUM"))
        ps_o = ctx.enter_context(tc.tile_pool(name="ps_o", bufs=2, space="PSUM"))

        X, E, RSP, RRE, WVP, EPT = {}, {}, {}, {}, {}, {}

        def load_x(b):
            # one casting DMA per XH-wide chunk: [128, KC, XH] bf16
            X[b] = []
            for h in range(NH):
                t = xpool.tile([128, KC, XH], BF16, tag="xt", name=f"x{b}_{h}")
                src = x_d[b, :, :, h * XH:(h + 1) * XH].rearrange("k p n -> p k n")
                nc.gpsimd.dma_start(t[:], src)
                X[b].append(t)

        # x first (SWDGE queue head = critical path); weights go over HWDGE
        # as f32 and are converted on DVE (wk) / inside the wvp scale (wv).
        load_x(0)
        load_x(1)

        wk_sb = []
        for kc in range(KC):
            f = wpool.tile([128, M], F32, tag=f"wkf{kc}", name=f"wkf{kc}")
            nc.sync.dma_start(f[:], wkT_d[kc * 128:(kc + 1) * 128, :])
            t = wpool.tile([128, M], BF16, tag=f"wk{kc}", name=f"wk{kc}")
            nc.vector.tensor_copy(t[:], f[:])
            wk_sb.append(t)
        wv_sb = []
        for km in range(KM):
            t = wpool.tile([128, C], F32, tag=f"wv{km}", name=f"wv{km}")
            nc.sync.dma_start(t[:], wvT_d[km * 128:(km + 1) * 128, :])
            wv_sb.append(t)

        def xs(b, kc, j):
            h, jj = j // (XH // NT), j % (XH // NT)
            return X[b][h][:, kc, jj * NT:(jj + 1) * NT]

        def init_A(b):
            E[b] = [epool.tile([128, N], BF16, tag="e", name=f"e{b}_{km}")
                    for km in range(KM)]
            RSP[b] = [spool.tile([128, NJ], F32, tag="rsp", name=f"rsp{b}_{km}")
                      for km in range(KM)]

        def emit_A(b, j):
            for km in range(KM):
                pl = ps_l.tile([128, NT], F32, tag="pl", name=f"pl{b}_{j}_{km}")
                for kc in range(KC):
                    nc.tensor.matmul(pl[:], wk_sb[kc][:, km * 128:(km + 1) * 128],
                                     xs(b, kc, j),
                                     start=(kc == 0), stop=(kc == KC - 1))
                nc.scalar.activation(E[b][km][:, j * NT:(j + 1) * NT], pl[:],
                                     AF.Exp, accum_out=RSP[b][km][:, j:j + 1])

        def emit_stats(b):
            RRE[b], WVP[b] = [], []
            for km in range(KM):
                rs = spool.tile([128, 1], F32, tag="rs", name=f"rs{b}_{km}")
                nc.vector.tensor_reduce(rs[:], RSP[b][km][:], axis=AX.X, op=ALU.add)
                rr = spool.tile([128, 1], F32, tag="rr", name=f"rr{b}_{km}")
                nc.vector.reciprocal(rr[:], rs[:])
                rrb = spool.tile([128, 1], BF16, tag="rrb", name=f"rrb{b}_{km}")
                nc.vector.tensor_copy(rrb[:], rr[:])
                RRE[b].append(rrb)
                t = wvp_pool.tile([128, C], BF16, tag="wvp", name=f"wvp{b}_{km}")
                nc.vector.tensor_scalar_mul(t[:], wv_sb[km][:], rr[:])
                WVP[b].append(t)

        def emit_chain(b, j):
            cs = ps_cs.tile([1, NT], F32, tag="cs", name=f"cs{b}_{j}")
            for km in range(KM):
                nc.tensor.matmul(cs[:], RRE[b][km][:],
                                 E[b][km][:, j * NT:(j + 1) * NT],
                                 start=(km == 0), stop=(km == KM - 1))
            # 1/cs as exp(-ln(cs)): Ln and Exp share one ACT table
            # (natural_log_exp_and_others), so no table swaps mid-kernel.
            lcs = bcpool.tile([1, NT], F32, tag="lcs", name=f"lcs{b}_{j}")
            nc.scalar.activation(lcs[:], cs[:], AF.Ln)
            rcs = bcpool.tile([1, NT], BF16, tag="rcs", name=f"rcs{b}_{j}")
            nc.scalar.activation(rcs[:], lcs[:], AF.Exp, scale=-1.0)
            bc = bcpool.tile([128, NT], BF16, tag="bc", name=f"bc{b}_{j}")
            nc.gpsimd.partition_broadcast(bc[:], rcs[:])
            ep_t = []
            for km in range(KM):
                t = eppool.tile([128, NT], BF16, tag="epp", name=f"epp{b}_{j}_{km}")
                nc.vector.tensor_tensor(t[:], E[b][km][:, j * NT:(j + 1) * NT],
                                        bc[:], op=ALU.mult)
                ep_t.append(t)
            EPT[(b, j)] = ep_t

        def emit_mm2_pair(b, j0):
            # MM2 + residual add + store for columns [j0*NT, (j0+2)*NT)
            ep0 = EPT.pop((b, j0))
            ep1 = EPT.pop((b, j0 + 1))
            h = j0 // (XH // NT)
            for co in range(KC):
                po = ps_o.tile([128, 2 * NT], F32, tag="po", name=f"po{b}_{j0}_{co}")
                for jj, ep_t in ((0, ep0), (1, ep1)):
                    for km in range(KM):
                        nc.tensor.matmul(po[:, jj * NT:(jj + 1) * NT],
                                         WVP[b][km][:, co * 128:(co + 1) * 128],
                                         ep_t[km][:],
                                         start=(km == 0), stop=(km == KM - 1))
                yt = ypool.tile([128, 2 * NT], F32, tag="y", name=f"y{b}_{j0}_{co}")
                nc.vector.tensor_tensor(yt[:], po[:], X[b][h][:, co, :],
                                        op=ALU.add)
                nc.sync.dma_start(
                    y_d[b, co, :, j0 * NT:(j0 + 2) * NT], yt[:])

        # ---- schedule ----
        init_A(0)
        for j in range(NJ):
            emit_A(0, j)
        emit_stats(0)
        for j in range(NJ):
            emit_chain(0, j)
        init_A(1)
        for j in range(NJ):
            emit_A(1, j)
            if j % 2 == 1:
                emit_mm2_pair(0, j - 1)
        emit_stats(1)
        for j in range(NJ):
            emit_chain(1, j)
            if j % 2 == 1:
                emit_mm2_pair(1, j - 1)
    return nc


_CACHE = {}


def _get_program():
    if "nc" not in _CACHE:
        nc = bacc.Bacc("TRN2", target_bir_lowering=False, debug=False,
                       enable_asserts=True)
        _build(nc)
        nc.compile()
        _CACHE["nc"] = nc
    return _CACHE["nc"]


def _in_maps(x, Wk, Wv):
    x = np.ascontiguousarray(np.asarray(x), dtype=np.float32)
    wkT = np.ascontiguousarray(np.asarray(Wk, dtype=np.float32).T)
    wvT = np.ascontiguousarray(np.asarray(Wv, dtype=np.float32).T)
    xs = x.reshape(NCORES, BPC, KC, 128, N)
    return [{"x": xs[i], "wkT": wkT, "wvT": wvT} for i in range(NCORES)]


def kernel(x, Wk, Wv):
    nc = _get_program()
    res = run_bass_kernel_spmd(nc, _in_maps(x, Wk, Wv), list(range(NCORES)))
    y = np.concatenate([res.results[i]["y"].reshape(BPC, C, N)
                        for i in range(NCORES)], axis=0)
    return np.ascontiguousarray(y, dtype=np.float32)


# revision 7
# speedup vs baseline: 1.0689x; 1.0689x over previous
"""External Attention (nn_External_Attention) on 8 TRN2 NeuronCores.

kernel(x, Wk, Wv) -> x + Wv @ l1norm_M(softmax_N(Wk @ x))
  x  [16, 512, 4096] f32,  Wk [256, 512] f32,  Wv [512, 256] f32

Sharding: data-parallel over batch B=16 -> 2 batches per core across 8 cores.
Each core runs an identical Bass/Tile program on its batch shard; results are
concatenated on host.

Per-core pipeline (C=512, M=256, N=4096), all-bf16 on the PE:
  x is loaded HBM->SBUF as bf16 via casting SWDGE (gpsimd) DMAs -- no engine
  pass for the conversion, half the SBUF residency of f32.
  phase A (per 512-col tile j): pl = Wk^T x (PE, bf16), E = exp(pl) (ACT,
      with per-tile row-sum accumulators)
  stats: r = sum_N E, rr = 1/r (DVE), Wv' = Wv^T * rr (bf16)
  chains (all j of a batch back-to-back, so ACT Exp<->Recip table swaps are
      batched: 3 total instead of one per j):
      cs = rr^T E (PE), rcs = 1/cs (ACT raw table recip, bf16 out),
      bc = partition_broadcast(rcs) (GPSIMD), E' = E*bc (DVE, 2x bf16 mode)
  phase B (per j-pair, interleaved with phase A of the next batch to keep
      the PE and DMA continuously busy):
      po[128,1024] = Wv'^T E' (PE, bf16, two PSUM banks),
      y = po + x_bf16 (DVE), y -> HBM (HWDGE on SP)

End-to-end relative L2 error vs the fp32 reference: ~1e-3 (bf16 x add).
"""
from contextlib import ExitStack

import numpy as np

import concourse.bacc as bacc
import concourse.mybir as mybir
import concourse.tile as tile
from concourse.bass_utils import run_bass_kernel_spmd

F32 = mybir.dt.float32
BF16 = mybir.dt.bfloat16
AF = mybir.ActivationFunctionType
ALU = mybir.AluOpType
AX = mybir.AxisListType

B, C, M, N = 16, 512, 256, 4096
NCORES = 8
BPC = B // NCORES
NT = 512
NJ = N // NT          # 8 column tiles
KC = C // 128         # 4
KM = M // 128         # 2
XH = 1024             # x load chunk width (one chunk covers 2 j tiles)
NH = N // XH          # 4


def _build(nc):
    x_d = nc.dram_tensor("x", [BPC, KC, 128, N], F32, kind="ExternalInput").ap()
    wkT_d = nc.dram_tensor("wkT", [C, M], F32, kind="ExternalInput").ap()
    wvT_d = nc.dram_tensor("wvT", [M, C], F32, kind="ExternalInput").ap()
    y_d = nc.dram_tensor("y", [BPC, KC, 128, N], F32, kind="ExternalOutput").ap()

    with tile.TileContext(nc) as tc, ExitStack() as ctx:
        wpool = ctx.enter_context(tc.tile_pool(name="w", bufs=1))
        xpool = ctx.enter_context(tc.tile_pool(name="xp", bufs=2 * NH))
        epool = ctx.enter_context(tc.tile_pool(name="ep", bufs=2 * KM))
        eppool = ctx.enter_context(tc.tile_pool(name="epp", bufs=18))
        spool = ctx.enter_context(tc.tile_pool(name="sp", bufs=4))
        wvp_pool = ctx.enter_context(tc.tile_pool(name="wvp", bufs=2 * KM))
        ypool = ctx.enter_context(tc.tile_pool(name="yp", bufs=4))
        bcpool = ctx.enter_context(tc.tile_pool(name="bcp", bufs=10))
        ps_l = ctx.enter_context(tc.tile_pool(name="ps_l", bufs=2, space="PSUM"))
        ps_cs = ctx.enter_context(tc.tile_pool(name="ps_cs", bufs=2, space="PSUM"))
        ps_o = ctx.enter_context(tc.tile_pool(name="ps_o", bufs=2, space="PSUM"))

        X, E, RSP, RRE, WVP, EPT = {}, {}, {}, {}, {}, {}

        def load_x(b):
            # one casting DMA per XH-wide chunk: [128, KC, XH] bf16
            X[b] = []
            for h in range(NH):
                t = xpool.tile([128, KC, XH], BF16, tag="xt", name=f"x{b}_{h}")
                src = x_d[b, :, :, h * XH:(h + 1) * XH].rearrange("k p n -> p k n")
                nc.gpsimd.dma_start(t[:], src)
                X[b].append(t)

        # x first (SWDGE queue head = critical path); weights go over HWDGE
        # as f32 and are converted on DVE (wk) / inside the wvp scale (wv).
        load_x(0)
        load_x(1)

        wk_sb = []
        for kc in range(KC):
            f = wpool.tile([128, M], F32, tag=f"wkf{kc}", name=f"wkf{kc}")
            nc.sync.dma_start(f[:], wkT_d[kc * 128:(kc + 1) * 128, :])
            t = wpool.tile([128, M], BF16, tag=f"wk{kc}", name=f"wk{kc}")
            nc.vector.tensor_copy(t[:], f[:])
            wk_sb.append(t)
        wv_sb = []
        for km in range(KM):
            t = wpool.tile([128, C], F32, tag=f"wv{km}", name=f"wv{km}")
            nc.sync.dma_start(t[:], wvT_d[km * 128:(km + 1) * 128, :])
            wv_sb.append(t)

        def xs(b, kc, j):
            h, jj = j // (XH // NT), j % (XH // NT)
            return X[b][h][:, kc, jj * NT:(jj + 1) * NT]

        def init_A(b):
            E[b] = [epool.tile([128, N], BF16, tag="e", name=f"e{b}_{km}")
                    for km in range(KM)]
            RSP[b] = [spool.tile([128, NJ], F32, tag="rsp", name=f"rsp{b}_{km}")
                      for km in range(KM)]

        def emit_A(b, j):
            for km in range(KM):
                pl = ps_l.tile([128, NT], F32, tag="pl", name=f"pl{b}_{j}_{km}")
                for kc in range(KC):
                    nc.tensor.matmul(pl[:], wk_sb[kc][:, km * 128:(km + 1) * 128],
                                     xs(b, kc, j),
                                     start=(kc == 0), stop=(kc == KC - 1))
                nc.scalar.activation(E[b][km][:, j * NT:(j + 1) * NT], pl[:],
                                     AF.Exp, accum_out=RSP[b][km][:, j:j + 1])

        def emit_stats(b):
            RRE[b], WVP[b] = [], []
            for km in range(KM):
                rs = spool.tile([128, 1], F32, tag="rs", name=f"rs{b}_{km}")
                nc.vector.tensor_reduce(rs[:], RSP[b][km][:], axis=AX.X, op=ALU.add)
                rr = spool.tile([128, 1], F32, tag="rr", name=f"rr{b}_{km}")
                nc.vector.reciprocal(rr[:], rs[:])
                rrb = spool.tile([128, 1], BF16, tag="rrb", name=f"rrb{b}_{km}")
                nc.vector.tensor_copy(rrb[:], rr[:])
                RRE[b].append(rrb)
                t = wvp_pool.tile([128, C], BF16, tag="wvp", name=f"wvp{b}_{km}")
                nc.vector.tensor_scalar_mul(t[:], wv_sb[km][:], rr[:])
                WVP[b].append(t)

        def emit_chain(b, j):
            cs = ps_cs.tile([1, NT], F32, tag="cs", name=f"cs{b}_{j}")
            for km in range(KM):
                nc.tensor.matmul(cs[:], RRE[b][km][:],
                                 E[b][km][:, j * NT:(j + 1) * NT],
                                 start=(km == 0), stop=(km == KM - 1))
            # 1/cs as exp(-ln(cs)): Ln and Exp share one ACT table
            # (natural_log_exp_and_others), so no table swaps mid-kernel.
            lcs = bcpool.tile([1, NT], F32, tag="lcs", name=f"lcs{b}_{j}")
            nc.scalar.activation(lcs[:], cs[:], AF.Ln)
            rcs = bcpool.tile([1, NT], BF16, tag="rcs", name=f"rcs{b}_{j}")
            nc.scalar.activation(rcs[:], lcs[:], AF.Exp, scale=-1.0)
            bc = bcpool.tile([128, NT], BF16, tag="bc", name=f"bc{b}_{j}")
            nc.gpsimd.partition_broadcast(bc[:], rcs[:])
            ep_t = []
            for km in range(KM):
                t = eppool.tile([128, NT], BF16, tag="epp", name=f"epp{b}_{j}_{km}")
                nc.vector.tensor_tensor(t[:], E[b][km][:, j * NT:(j + 1) * NT],
                                        bc[:], op=ALU.mult)
                ep_t.append(t)
            EPT[(b, j)] = ep_t

        def emit_mm2_pair(b, j0):
            # MM2 + residual add + store for columns [j0*NT, (j0+2)*NT)
            ep0 = EPT.pop((b, j0))
            ep1 = EPT.pop((b, j0 + 1))
            h = j0 // (XH // NT)
            for co in range(KC):
                po = ps_o.tile([128, 2 * NT], F32, tag="po", name=f"po{b}_{j0}_{co}")
                for jj, ep_t in ((0, ep0), (1, ep1)):
                    for km in range(KM):
                        nc.tensor.matmul(po[:, jj * NT:(jj + 1) * NT],
                                         WVP[b][km][:, co * 128:(co + 1) * 128],
                                         ep_t[km][:],
                                         start=(km == 0), stop=(km == KM - 1))
                yt = ypool.tile([128, 2 * NT], F32, tag="y", name=f"y{b}_{j0}_{co}")
                nc.vector.tensor_tensor(yt[:], po[:], X[b][h][:, co, :],
                                        op=ALU.add)
                nc.sync.dma_start(
                    y_d[b, co, :, j0 * NT:(j0 + 2) * NT], yt[:])

        # ---- schedule ----
        init_A(0)
        for j in range(NJ):
            emit_A(0, j)
        emit_stats(0)
        for j in range(NJ):
            emit_chain(0, j)
        init_A(1)
        for j in range(NJ):
            emit_A(1, j)
            if j % 2 == 1:
                emit_mm2_pair(0, j - 1)
        emit_stats(1)
        for j in range(NJ):
            emit_chain(1, j)
            if j % 2 == 1:
                emit_mm2_pair(1, j - 1)
    return nc


_CACHE = {}


def _steer_act_tables():
    """Make the act-table placement pass resolve both Exp and Ln to the one
    table that holds them both (natural_log_exp_and_others), instead of
    thrashing between exp_and_others and natural_log on every chain.

    Only the *advertised* function sets of the two greedy-first tables are
    filtered; list order (and hence act_func_set_id numbering) is untouched,
    so the runtime still loads real, correct tables.
    """
    from concourse import hw_specs

    orig = hw_specs.get_activation_tables

    def patched(arch):
        tabs = dict(orig(arch))
        exp_f = mybir.ActivationFunctionType.Exp
        ln_f = mybir.ActivationFunctionType.Ln
        both = {n for n, s in tabs.items() if exp_f in s and ln_f in s}
        if both:
            tabs = {n: (s - {exp_f, ln_f} if n not in both else s)
                    for n, s in tabs.items()}
        return tabs

    bacc.get_activation_tables = patched
    return orig


def _get_program():
    if "nc" not in _CACHE:
        nc = bacc.Bacc("TRN2", target_bir_lowering=False, debug=False,
                       enable_asserts=True)
        _build(nc)
        orig = _steer_act_tables()
        try:
            nc.compile()
        finally:
            bacc.get_activation_tables = orig
        _CACHE["nc"] = nc
    return _CACHE["nc"]


def _in_maps(x, Wk, Wv):
    x = np.ascontiguousarray(np.asarray(x), dtype=np.float32)
    wkT = np.ascontiguousarray(np.asarray(Wk, dtype=np.float32).T)
    wvT = np.ascontiguousarray(np.asarray(Wv, dtype=np.float32).T)
    xs = x.reshape(NCORES, BPC, KC, 128, N)
    return [{"x": xs[i], "wkT": wkT, "wvT": wvT} for i in range(NCORES)]


def kernel(x, Wk, Wv):
    nc = _get_program()
    res = run_bass_kernel_spmd(nc, _in_maps(x, Wk, Wv), list(range(NCORES)))
    y = np.concatenate([res.results[i]["y"].reshape(BPC, C, N)
                        for i in range(NCORES)], axis=0)
    return np.ascontiguousarray(y, dtype=np.float32)
